# revision 59
# baseline (speedup 1.0000x reference)
"""Trainium2 Bass kernel for nn_CustomNetworkGINSeroMean (GIN message passing +
TopK pooling + SERO readout + BN/FC head).

Strategy (data-parallel over batch B=64, 8 graphs per NeuronCore):
  - Dense alive-mask pooling (no gathers); graph state stays in SBUF.
  - Host pre-transposes pos and the layer-0 normalized adjacency, so layer 0
    starts matmuls straight off the input DMAs (no on-device transposes).
  - hT = x^T @ adjnT + x^T @ I via accumulating PE matmuls.
  - Generated weights never materialized: G = h @ W2om (o-major column order)
    with 9 extra host-precomputed score columns (W2e @ pw), so the topk score
    is a single fused multiply-reduce off G -- it does not wait on the
    combine.  Combine = one broadcast multiply + one grouped reduce per graph.
  - Last layer skips the combine entirely: r comes straight from
    scl-weighted prd via 1-row PE matmuls, so the final AllGather triggers
    as early as possible.  Earlier layers take r from xo (needed for xn
    anyway) with ONE batched [8, 512] matmul whose block diagonal holds all
    graphs' r; tiny per-graph DMAs lift the diagonal into the gather input.
  - Gather payload is [BL, H] so the gathered [B, H] block is contiguous;
    one PE transpose on the tail side replaces a strided 512-chunk DMA.
  - ACT table: sigmoid_and_others (sigmoid + erf + tanh).  Sigmoid and exact
    erf-gelu are single ACT ops; BN uses bn_stats/bn_aggr + quake rsqrt.
"""

import numpy as np

import concourse.bass as bass
import concourse.tile as tile
from concourse import bacc, mybir
from concourse.bass_utils import run_bass_kernel_spmd

F32 = mybir.dt.float32
F32R = mybir.dt.float32r
I32 = mybir.dt.int32
AF = mybir.ActivationFunctionType
ALU = mybir.AluOpType
AX = mybir.AxisListType

B, R, D = 64, 100, 100
H = 64
K = 8
KE = K + 1
FC = (64, 32)
NCLASS = 2
NL = 3
NCORES = 8
BL = B // NCORES
MS = (50, 25, 13)
DIN = (100, 64, 64)
NEG = -1.0e30
EPS_BN = 1e-5
RSQ2 = 0.7071067811865476
W2A = 56 * KE          # G columns for o in [0,56)
W2B = 8 * KE + KE + 1  # G cols for o in [56,64) + 9 score cols + 1 pad (even N)
PCOL = 8 * KE          # offset of score cols inside the gB tile

TRACE = False
_CACHE = {}


def _wcols():
    cols = {}
    off = 0

    def put(name, w):
        nonlocal off
        cols[name] = (off, w)
        off += w

    # hot block (layer-0 critical path): w1s + w2om_0
    for l in range(NL):
        put(f"w1_{l}", K)
    put("w2om_0", W2A + W2B)
    # cold block
    for l in range(1, NL):
        put(f"w2om_{l}", W2A + W2B)
    for l in range(NL):
        put(f"sew_{l}", H)
    for l in range(NL):
        put(f"saw_{l}", H)      # pre-scaled by 0.5 (erf-gelu factor)
    put("fcw0", NL * FC[0])
    put("fcw1", FC[1])
    put("fw", NCLASS)
    for l in range(NL):
        put(f"sbg_{l}", 1)
        put(f"sbb_{l}", 1)
        put(f"sab_{l}", 1)
    for nm in ("fcb0", "bng0", "bnb0", "fcb1", "bng1", "bnb1", "fb"):
        put(nm, 1)
    return cols, off


WCOLS, WTOT = _wcols()
CSPLIT = WCOLS["w2om_1"][0]  # hot wpack = [0, CSPLIT)
CB0 = WCOLS["sbg_0"][0]      # first 1-wide constant column


def _emit(tc, io):
    nc = tc.nc
    consts = io["consts_pool"]
    state = io["state_pool"]
    work = io["work_pool"]
    psum = io["psum_pool"]
    dram = io["dram_pool"]

    # ---- input DMAs, spread across engine queues ----
    # wpA + posT lead their queues: the first PE ops (U matmuls) wait on them.
    wpA = consts.tile([128, CSPLIT], F32R, tag="wpA")
    nc.sync.dma_start(wpA[:], io["wpackA"][:])
    posT = consts.tile([R, BL, R], F32R, tag="posT")
    nc.scalar.dma_start(posT[:], io["posT"][:])
    adjnT0 = consts.tile([R, BL, R], F32R, tag="adjnT0")
    nc.sync.dma_start(adjnT0[:], io["adjnT0"][:])
    ipk = consts.tile([128, 256], F32R, tag="ipk")
    nc.scalar.dma_start(ipk[:], io["ipack"][:])
    wpB = consts.tile([128, WTOT - CSPLIT], F32R, tag="wpB")

    zc = consts.tile([128, 1], F32, tag="zc")
    nc.vector.memset(zc[:], 0.0)
    magicc = consts.tile([128, 1], I32, tag="magicc")
    nc.vector.memset(magicc[:], 0x5F3759DF)


    def idR(p):
        return ipk[0:p, 0:p]                # f32r view for f32r transposes

    def idF(p):
        return idR(p).bitcast(F32)          # fp32 view for fp32 transposes

    def notIv(p):
        return ipk[0:p, 128 : 128 + p].bitcast(F32)

    # preload the sigmoid/erf/tanh ACT table set under the DMA wait
    dume = work.tile([1, 1], F32, tag="dume")
    nc.scalar.activation(dume[:], zc[0:1, 0:1], AF.Sigmoid, bias=zc[0:1, 0:1])


    # dummy warm-up collective: absorbs the CC rendezvous barrier during the
    # load phase so the real gathers start without delay
    dcl = dram.tile([1, 1], F32, tag="dcl")
    nc.sync.dma_start(dcl[:], zc[0:1, 0:1])
    dcg = dram.tile([NCORES, 1, 1], F32, tag="dcg")
    nc.gpsimd.collective_compute(
        "AllGather",
        ALU.bypass,
        replica_groups=[list(range(NCORES))],
        ins=[dcl[:].opt()],
        outs=[dcg[:].opt()],
    )
    # x and raw adj on the gpsimd queue, behind the warm-up trigger.
    # wpB (cold: first needed at layer 1) goes last here so the scalar
    # queue's completion counter -- which gates the first PE matmuls --
    # is not held back by its long transfer.
    x0 = state.tile([R, BL * R], F32R, tag="x0")
    nc.gpsimd.dma_start(x0[:].rearrange("r (g c) -> r g c", g=BL), io["x"][:])
    adj = state.tile([R, BL, R], F32, tag="adj")
    nc.gpsimd.dma_start(adj[:], io["adj"][:])
    nc.gpsimd.dma_start(wpB[:], io["wpackB"][:])

    def wsl(name, p, c0=0, w=None):
        off, width = WCOLS[name]
        if w is None:
            w = width - c0
        if off < CSPLIT:
            return wpA[0:p, off + c0 : off + c0 + w]
        return wpB[0:p, off - CSPLIT + c0 : off - CSPLIT + c0 + w]

    # fp32 view of the per-feature column constants (ts scalars must be fp32)
    colsF = consts.tile([128, WTOT - CB0], F32, tag="colsF")
    nc.vector.tensor_copy(colsF[:], wpB[:, CB0 - CSPLIT :])

    def wslF(name, p):
        off, width = WCOLS[name]
        return colsF[0:p, off - CB0 : off - CB0 + width]

    # ---- U_l = relu(posT^T @ w1_l); layer 0 up front, layers 1-2 are
    # emitted inside the previous layer's topk window (PE is idle there) ----
    ues = []

    def emit_ue(l):
        up = psum.tile([R, BL, K], F32, tag="gB", bufs=2)
        for g in range(BL):
            nc.tensor.matmul(up[:, g, :], posT[:, g, :], wsl(f"w1_{l}", R))
        ue = state.tile([R, BL * KE], F32, tag=f"ue{l}")
        uev = ue[:].rearrange("r (g k) -> r g k", k=KE)
        nc.scalar.activation(uev[:, :, 0:K], up[:], AF.Relu, bias=zc[0:R, 0:1])
        nc.vector.memset(uev[:, :, K:KE], 1.0)
        ues.append(ue)

    emit_ue(0)
    emit_ue(1)
    emit_ue(2)

    def uecols(l, g):
        return ues[l][:, g * KE : (g + 1) * KE]

    # ---- head helpers ----
    rfs = []
    seros = []

    def quake_mul(P, lv, gcol):
        """gr = rsqrt(lv) * gcol via quake + 1 Newton step (all DVE)."""
        yi = work.tile([P, 1], I32, tag="byi")
        nc.vector.tensor_scalar(
            yi[:], lv.bitcast(I32), 1, None, ALU.logical_shift_right
        )
        nc.vector.tensor_tensor(yi[:], magicc[0:P, :], yi[:], ALU.subtract)
        yv = yi[:].bitcast(F32)
        t1 = work.tile([P, 1], F32, tag="bt1")
        nc.vector.tensor_tensor(t1[:], yv, yv, ALU.mult)
        nc.vector.tensor_tensor(t1[:], t1[:], lv, ALU.mult)
        nc.vector.tensor_scalar(t1[:], t1[:], -0.5, 1.5, ALU.mult, ALU.add)
        gr = work.tile([P, 1], F32, tag="bgr")
        nc.vector.scalar_tensor_tensor(gr[:], yv, t1[:], gcol, ALU.mult, ALU.mult)
        return gr

    def bn_apply(z, gcol, bcolF, P):
        """zn = (z - mu)/sqrt(var+eps) * g + b over the batch (free) axis."""
        st6 = work.tile([P, 6], F32, tag="bst")
        nc.vector.bn_stats(st6[:], z[:])
        mv = work.tile([P, 2], F32, tag="bmv")
        nc.vector.bn_aggr(mv[:], st6[:])
        lv = work.tile([P, 1], F32, tag="blv")
        nc.vector.tensor_scalar(lv[:], mv[:, 1:2], EPS_BN, None, ALU.add)
        gr = quake_mul(P, lv[:], gcol)
        zn = work.tile([P, B], F32, tag="bzn")
        nc.vector.scalar_tensor_tensor(
            zn[:], z[:], mv[:, 0:1], gr[:].broadcast_to([P, B]),
            ALU.subtract, ALU.mult,
        )
        znb = work.tile([P, B], F32R, tag="bznb")
        nc.vector.tensor_scalar(znb[:], zn[:], bcolF, None, ALU.add)
        return znb

    def emit_sero(l):
        # rf: transpose the contiguous [B, H] gathered block
        rfp = psum.tile([H, B], F32, tag="tp", bufs=2)
        nc.tensor.transpose(rfp[:], rfs[l][:], idF(B))
        rf = work.tile([H, B], F32R, tag=f"rf{l}")
        nc.vector.tensor_copy(rf[:], rfp[:])
        z1 = psum.tile([H, B], F32, tag="ht", bufs=2)
        nc.tensor.matmul(z1[:], wsl(f"sew_{l}", H), rf[:])
        znb = bn_apply(z1, wsl(f"sbg_{l}", H), wslF(f"sbb_{l}", H), H)
        er = work.tile([H, B], F32, tag="ger")
        nc.scalar.activation(er[:], znb[:], AF.Erf, scale=RSQ2, bias=zc[0:H, 0:1])
        e2 = work.tile([H, B], F32R, tag="ge2")
        nc.vector.scalar_tensor_tensor(
            e2[:], er[:], 1.0, znb[:], ALU.add, ALU.mult
        )
        ap_ = psum.tile([H, B], F32, tag="tp", bufs=2)
        nc.tensor.matmul(ap_[:], wsl(f"saw_{l}", H), e2[:])
        att = work.tile([H, B], F32, tag="att")
        nc.scalar.activation(att[:], ap_[:], AF.Sigmoid, bias=wslF(f"sab_{l}", H))
        sero = work.tile([H, B], F32R, tag=f"sero{l}")
        nc.vector.tensor_tensor(sero[:], rf[:], att[:], ALU.mult)
        seros.append(sero)

    xcur = x0
    aliveT = None

    for l in range(NL):
        din, m = DIN[l], MS[l]
        last = l == NL - 1

        # ---- normalized adjacency (+I folded in) for l>0; emitted in
        # half-batches so graphs 0-3 unblock while the previous layer's
        # augmentation is still finishing graphs 4-7 ----
        if l > 0:
            deg = work.tile([R, BL], F32, tag="deg")
            invd = work.tile([R, BL], F32, tag="invd")
            adjn = work.tile([R, BL, R], F32, tag="adjn")
            for h0, h1 in ((0, 4), (4, BL)):
                nc.vector.tensor_reduce(
                    deg[:, h0:h1], adj[:, h0:h1, :], AX.X, ALU.add
                )
                nc.vector.tensor_scalar_max(deg[:, h0:h1], deg[:, h0:h1], 1e-12)
                nc.vector.reciprocal(invd[:, h0:h1], deg[:, h0:h1])
                nc.gpsimd.tensor_tensor(
                    adjn[:, h0:h1, :], adj[:, h0:h1, :],
                    invd[:, h0:h1].unsqueeze(2).broadcast_to([R, h1 - h0, R]),
                    ALU.mult,
                )
                nc.gpsimd.tensor_tensor(
                    adjn[:, h0:h1, :], adjn[:, h0:h1, :],
                    idF(R).unsqueeze(1).broadcast_to([R, h1 - h0, R]), ALU.add,
                )
            adjnT = work.tile([R, BL * R], F32R, tag="adjnT")

        # ---- per-graph: hT, G (o-major + score cols), prd, score ----
        hT = work.tile([din, BL * R], F32R, tag="hT")
        sCol = work.tile([R, BL], F32, tag="sCol")
        prds = []
        for g in range(BL):
            if l > 0:
                tp = psum.tile([R, R], F32, tag="tp", bufs=2)
                nc.tensor.transpose(tp[:], adjn[:, g, :], idF(R))
                adjnT_g = adjnT[:, g * R : (g + 1) * R]
                nc.scalar.copy(adjnT_g, tp[:])
            else:
                adjnT_g = adjnT0[:, g, :]
            xg = xcur[:, g * din : (g + 1) * din]
            ht = psum.tile([din, R], F32, tag="ht", bufs=2)
            nc.tensor.matmul(ht[:], xg, adjnT_g)
            hts = hT[:, g * R : (g + 1) * R]
            nc.scalar.copy(hts, ht[:])
            gA = psum.tile([R, W2A], F32, tag="gA", bufs=2)
            gB = psum.tile([R, W2B], F32, tag="gB", bufs=2)
            nc.tensor.matmul(gA[:], hts, wsl(f"w2om_{l}", din, 0, W2A))
            nc.tensor.matmul(gB[:], hts, wsl(f"w2om_{l}", din, W2A, W2B))
            # score: s[n] = sum_k U[n,k] * P[n,k]  (one fused DVE op)
            tjk = work.tile([R, KE], F32, tag="tjk")
            nc.vector.scalar_tensor_tensor(
                tjk[:], gB[:, PCOL : PCOL + KE], 1.0, uecols(l, g),
                ALU.mult, ALU.mult, accum_out=sCol[:, g : g + 1],
            )
            # prd = G * U (o-major broadcast); combine finishes after topk.
            # Graphs 5-7 run the big multiply on gpsimd to offload the DVE.
            prd = work.tile([R, H, KE], F32R, tag=f"prd{g}")
            ueb = uecols(l, g).unsqueeze(1)
            eng = nc.vector
            eng.tensor_tensor(
                prd[:, 0:56, :], gA[:].rearrange("r (o k) -> r o k", k=KE),
                ueb.broadcast_to([R, 56, KE]), ALU.mult,
            )
            eng.tensor_tensor(
                prd[:, 56:64, :],
                gB[:, 0:PCOL].rearrange("r (o k) -> r o k", k=KE),
                ueb.broadcast_to([R, 8, KE]), ALU.mult,
            )
            prds.append(prd)

        # sigmoid(score) for the value-scaling (topk itself uses raw scores)
        sig = work.tile([R, BL], F32, tag="sig")
        nc.scalar.activation(sig[:], sCol[:], AF.Sigmoid, bias=zc[0:R, 0:1])

        # ---- topk selection (graph-major, on raw scores) ----
        st = psum.tile([BL, R], F32, tag="tp", bufs=2)
        nc.tensor.transpose(st[:], sCol[:], idF(R))
        sm = work.tile([BL, R], F32, tag="smk")
        if aliveT is None:
            nc.vector.tensor_copy(sm[:], st[:])
        else:
            nc.vector.tensor_tensor(sm[:], st[:], aliveT[:], ALU.mult)
            nc.vector.tensor_tensor(sm[:], sm[:], penT[:], ALU.add)
        wk = work.tile([BL, R], F32, tag="wk")
        nc.vector.tensor_copy(wk[:], sm[:])
        for t in range((m + 7) // 8):
            mx = work.tile([BL, 8], F32, tag="mx")
            nc.vector.max(mx[:], wk[:])
            rem = m - 8 * t
            if rem < 8:
                nc.vector.memset(mx[:, rem:8], NEG)
            nc.vector.match_replace(wk[:], mx[:], wk[:], NEG)
        nmT = work.tile([BL, R], F32, tag=f"nmT{l}")
        nc.vector.tensor_tensor(nmT[:], sm[:], wk[:], ALU.subtract)
        nc.vector.tensor_scalar_min(nmT[:], nmT[:], 1.0)
        aliveT = nmT

        nmp = psum.tile([R, BL], F32, tag="tp", bufs=2)
        nc.tensor.transpose(nmp[:], nmT[:], idF(BL))
        nmCol = work.tile([R, BL], F32, tag="nmCol")
        nc.vector.tensor_copy(nmCol[:], nmp[:])
        sclC = work.tile([R, BL], F32, tag="sclC")
        nc.vector.tensor_tensor(sclC[:], sig[:], nmCol[:], ALU.mult)

        # ---- r_l, then AllGather immediately ----
        # Last layer: r straight from scl-weighted prd (PE) -- no combine at
        # all.  Other layers: combine first (xo is needed for xn anyway) and
        # take r from xo with cheap 1-row matmuls.
        sclR = work.tile([R, BL], F32R, tag="sclR")
        nc.vector.tensor_scalar_mul(sclR[:], sclC[:], 1.0 / m)
        rloc = dram.tile([1, BL * H], F32, tag=f"rloc{l}")
        if last:
            rr = work.tile([1, BL * H], F32, tag="rr")
            for g in range(BL):
                prdf = prds[g][:].rearrange("r o k -> r (o k)")
                sPA = psum.tile([1, W2A], F32, tag="gA", bufs=2)
                sPB = psum.tile([1, PCOL], F32, tag="gB", bufs=2)
                nc.tensor.matmul(sPA[:], sclR[:, g : g + 1], prdf[:, 0:W2A])
                nc.tensor.matmul(sPB[:], sclR[:, g : g + 1], prdf[:, W2A:])
                nc.vector.tensor_reduce(
                    rr[0:1, g * H : g * H + 56],
                    sPA[:].rearrange("a (o k) -> a o k", k=KE), AX.X, ALU.add,
                )
                nc.vector.tensor_reduce(
                    rr[0:1, g * H + 56 : g * H + 64],
                    sPB[:].rearrange("a (o k) -> a o k", k=KE), AX.X, ALU.add,
                )
        else:
            # t1 + the augmentation's PE/ACT work go first so they overlap
            # the combine pools on the DVE; the notI masking is deferred past
            # the gather trigger (agp bufs=8 keeps all products alive).
            t1 = work.tile([R, BL, R], F32, tag="t1")
            for h0, h1 in ((0, 4), (4, BL)):
                nc.gpsimd.tensor_tensor(
                    t1[:, h0:h1, :], adj[:, h0:h1, :],
                    nmCol[:, h0:h1].unsqueeze(2).broadcast_to([R, h1 - h0, R]),
                    ALU.mult,
                )
            amT = work.tile([R, BL * R], F32R, tag="amT")
            am = work.tile([R, BL * R], F32R, tag="am")
            xo = work.tile([R, BL * H], F32R, tag="xo")
            rr = work.tile([BL, BL * H], F32, tag="rr")
            rtb = psum.tile([BL, BL * H], F32, tag="gB", bufs=2)
            for g in range(BL):
                tp = psum.tile([R, R], F32, tag="tp", bufs=2)
                nc.tensor.transpose(tp[:], t1[:, g, :], idF(R))
                nc.tensor.matmul(
                    tp[:], idF(R), idF(R), start=False, stop=True,
                    skip_group_check=True,
                )
                nc.scalar.mul(
                    amT[:, g * R : (g + 1) * R], tp[:], nmCol[:, g : g + 1]
                )
                ap2 = psum.tile([R, R], F32R, tag="ht", bufs=2)
                nc.tensor.transpose(ap2[:], amT[:, g * R : (g + 1) * R], idR(R))
                nc.scalar.copy(am[:, g * R : (g + 1) * R], ap2[:])
                agp = psum.tile([R, R], F32, tag="gA", bufs=2)
                nc.tensor.matmul(
                    agp[:], amT[:, g * R : (g + 1) * R],
                    am[:, g * R : (g + 1) * R],
                )
                # DVE: pool-g then notI-g, zipped so the aug PE/ACT chain is
                # hidden under the combine pools without stalling on agp reuse
                nc.vector.tensor_reduce(
                    xo[:, g * H : (g + 1) * H], prds[g][:], AX.X, ALU.add
                )
                nc.vector.tensor_tensor(adj[:, g, :], agp[:], notIv(R), ALU.mult)
                # batched r matmul in two halves: all graphs' r lands on the
                # block diagonal of [8, 512] (off-diagonal = cross-graph
                # junk); tiny DMAs lift the diagonal straight into rloc.
                # Half 1 fires after pool g=3 so it overlaps pools 4-7.
                if g == 3 or g == BL - 1:
                    c0, c1 = (0, 4 * H) if g == 3 else (4 * H, BL * H)
                    nc.tensor.matmul(
                        rtb[:, c0:c1], sclR[:, 0:BL], xo[:, c0:c1]
                    )
                    nc.vector.tensor_copy(rr[:, c0:c1], rtb[:, c0:c1])
                    for gg in range(c0 // H, c1 // H):
                        nc.sync.dma_start(
                            rloc[0:1, gg * H : (gg + 1) * H],
                            rr[gg : gg + 1, gg * H : (gg + 1) * H],
                        )
        if last:
            nc.sync.dma_start(rloc[:], rr[:])
        rg = dram.tile([NCORES, 1, BL * H], F32, tag=f"rg{l}")
        nc.gpsimd.collective_compute(
            "AllGather",
            ALU.bypass,
            replica_groups=[list(range(NCORES))],
            ins=[rloc[:].opt()],
            outs=[rg[:].opt()],
        )
        rfB = state.tile([B, H], F32, tag=f"rfB{l}")
        nc.sync.dma_start(rfB[:], rg[:].rearrange("c a (g h) -> (c a g) h", h=H))
        rfs.append(rfB)

        if last:
            # gather-2 flight time absorbs SERO-0/1 + the l=0,1 share of fc1
            emit_sero(0)
            emit_sero(1)
            f1 = psum.tile([FC[0], B], F32, tag="gB", bufs=2)
            for ll in range(2):
                nc.tensor.matmul(
                    f1[:], wsl("fcw0", H, ll * FC[0], FC[0]), seros[ll][:],
                    start=(ll == 0), stop=False,
                )
            io["f1"] = f1
            break

        # ---- pooled x for the next layer + deferred adjacency masking ----
        xn = state.tile([R, BL * H], F32R, tag=f"x{l + 1}")
        nc.gpsimd.tensor_tensor(
            xn[:].rearrange("r (g o) -> r g o", o=H),
            xo[:].rearrange("r (g o) -> r g o", o=H),
            sclC[:].unsqueeze(2).broadcast_to([R, BL, H]), ALU.mult,
        )
        # dead-node penalty for the NEXT layer's topk, off the hot path
        penT = work.tile([BL, R], F32, tag=f"pen{l}")
        nc.vector.tensor_scalar(penT[:], nmT[:], -1.0, -NEG, ALU.add, ALU.mult)

        xcur = xn

    # ---- tail: SERO of the last layer + FC head ----
    emit_sero(NL - 1)
    f1 = io["f1"]
    nc.tensor.matmul(
        f1[:], wsl("fcw0", H, 2 * FC[0], FC[0]), seros[2][:],
        start=False, stop=True,
    )
    z1h = work.tile([FC[0], B], F32, tag="z1h")
    nc.vector.tensor_scalar(z1h[:], f1[:], wslF("fcb0", FC[0]), 0.0, ALU.add, ALU.max)
    z1n = bn_apply(z1h, wsl("bng0", FC[0]), wslF("bnb0", FC[0]), FC[0])
    f2 = psum.tile([FC[1], B], F32, tag="ht", bufs=2)
    nc.tensor.matmul(f2[:], wsl("fcw1", FC[0]), z1n[:])
    z2h = work.tile([FC[1], B], F32, tag="z2h")
    nc.vector.tensor_scalar(z2h[:], f2[:], wslF("fcb1", FC[1]), 0.0, ALU.add, ALU.max)
    z2n = bn_apply(z2h, wsl("bng1", FC[1]), wslF("bnb1", FC[1]), FC[1])
    fo = psum.tile([NCLASS, B], F32, tag="tp", bufs=2)
    nc.tensor.matmul(fo[:], wsl("fw", FC[1]), z2n[:])
    outT = work.tile([NCLASS, B], F32, tag="outT")
    nc.vector.tensor_scalar(outT[:], fo[:], wslF("fb", NCLASS), 0.0, ALU.add, ALU.max)
    op = psum.tile([B, NCLASS], F32, tag="gA", bufs=2)
    nc.tensor.transpose(op[:], outT[:], idF(NCLASS))
    ofin = work.tile([B, NCLASS], F32, tag="ofin")
    nc.vector.tensor_copy(ofin[:], op[:])
    nc.sync.dma_start(io["out"][:], ofin[:])


def _build():
    nc = bacc.Bacc("TRN2", target_bir_lowering=False, debug=False, num_devices=NCORES)
    io = {}

    def dparam(name, shape, dtype=F32, kind="ExternalInput"):
        io[name] = nc.dram_tensor(name, list(shape), dtype, kind=kind).ap()

    dparam("x", (R, BL, R), F32R)
    dparam("adj", (R, BL, R))
    dparam("posT", (R, BL, R), F32R)
    dparam("adjnT0", (R, BL, R), F32R)
    dparam("wpackA", (128, CSPLIT), F32R)
    dparam("wpackB", (128, WTOT - CSPLIT), F32R)
    dparam("ipack", (128, 256), F32R)
    dparam("out", (B, NCLASS), kind="ExternalOutput")

    import contextlib

    with tile.TileContext(nc) as tc:
        with contextlib.ExitStack() as ctx:
            io["consts_pool"] = ctx.enter_context(tc.tile_pool(name="consts", bufs=1))
            io["state_pool"] = ctx.enter_context(tc.tile_pool(name="state", bufs=1))
            io["work_pool"] = ctx.enter_context(tc.tile_pool(name="work", bufs=2))
            io["psum_pool"] = ctx.enter_context(
                tc.tile_pool(name="psum", bufs=1, space="PSUM")
            )
            io["dram_pool"] = ctx.enter_context(
                tc.tile_pool(name="dram", bufs=1, space="DRAM")
            )
            with nc.allow_low_precision(reason="float32r is bit-identical fp32"):
                _emit(tc, io)
    nc.compile()
    return nc


def _prep_wpack(inputs):
    f = np.float32
    wpk = np.zeros((128, WTOT), f)

    def put(name, arr):
        off, w = WCOLS[name]
        arr = np.asarray(arr, f)
        if arr.ndim == 1:
            arr = arr.reshape(-1, 1)
        assert arr.shape[1] == w, (name, arr.shape, w)
        wpk[: arr.shape[0], off : off + w] = arr

    for l in range(NL):
        put(f"w1_{l}", inputs[f"w1_{l}"])
        din = DIN[l]
        w2 = np.asarray(inputs[f"w2_{l}"], f).reshape(K, din, H)
        b2 = np.asarray(inputs[f"b2_{l}"], f).reshape(1, din, H)
        w2e = np.concatenate([w2, b2], 0)                      # [KE, din, H]
        w2om = w2e.transpose(1, 2, 0).reshape(din, H * KE)     # [i, (o, k)]
        pw = np.asarray(inputs[f"pw_{l}"], f)
        pwn = pw / np.linalg.norm(pw)
        w2pw = np.einsum("kio,o->ik", w2e, pwn)                # [din, KE]
        pad = np.zeros((din, 1), f)
        put(f"w2om_{l}", np.concatenate([w2om, w2pw, pad], axis=1))
        put(f"sew_{l}", inputs[f"sew_{l}"])
        put(f"saw_{l}", 0.5 * np.asarray(inputs[f"saw_{l}"], f))
        put(f"sbg_{l}", inputs[f"sbg_{l}"])
        put(f"sbb_{l}", inputs[f"sbb_{l}"])
        put(f"sab_{l}", inputs[f"sab_{l}"])
    f0 = (
        np.asarray(inputs["fcw_0"], f)
        .reshape(NL, H, FC[0])
        .transpose(1, 0, 2)
        .reshape(H, NL * FC[0])
    )
    put("fcw0", f0)
    put("fcw1", inputs["fcw_1"])
    put("fw", inputs["fw"])
    put("fcb0", inputs["fcb_0"])
    put("bng0", inputs["bng_0"])
    put("bnb0", inputs["bnb_0"])
    put("fcb1", inputs["fcb_1"])
    put("bng1", inputs["bng_1"])
    put("bnb1", inputs["bnb_1"])
    put("fb", inputs["fb"])
    return wpk


def kernel(**inputs):
    inputs = {k: np.asarray(v) for k, v in inputs.items()}
    if "nc" not in _CACHE:
        _CACHE["nc"] = _build()
    nc = _CACHE["nc"]

    wpk = _prep_wpack(inputs)
    ipk = np.concatenate(
        [np.eye(128, dtype=np.float32), 1.0 - np.eye(128, dtype=np.float32)], axis=1
    )
    adj_f = np.asarray(inputs["adj"], np.float32)
    deg = np.maximum(adj_f.sum(-1, keepdims=True), 1e-12)
    adjn0i = adj_f / deg + np.eye(R, dtype=np.float32)
    in_maps = []
    for c in range(NCORES):
        s = slice(c * BL, (c + 1) * BL)
        in_maps.append(
            {
                "wpackA": np.ascontiguousarray(wpk[:, :CSPLIT]),
                "wpackB": np.ascontiguousarray(wpk[:, CSPLIT:]),
                "ipack": ipk,
                "x": np.ascontiguousarray(
                    inputs["x"][s].transpose(1, 0, 2), np.float32
                ),
                "adj": np.ascontiguousarray(adj_f[s].transpose(1, 0, 2)),
                "posT": np.ascontiguousarray(
                    np.asarray(inputs["pos"], np.float32)[s].transpose(2, 0, 1)
                ),
                "adjnT0": np.ascontiguousarray(adjn0i[s].transpose(2, 0, 1)),
            }
        )

    res = run_bass_kernel_spmd(
        nc, in_maps, core_ids=list(range(NCORES)), trace=TRACE
    )
    _CACHE["last_results"] = res
    return res.results[0]["out"]
